# revision 33
# baseline (speedup 1.0000x reference)
"""AdaLoRAWithBase distributed Trainium2 kernel (8 NeuronCores).

Reference computation (B=16, D=2048, ADA=1024, INTER=1024, R=8):
    h   = gelu(ada_emb @ w1 + b1)                  [B, INTER]
    xw  = h @ w2 + b2                              [B, 2*D*R]
    x_a = xw[:, :D*R]  -> [B, D, R]
    x_b = xw[:, D*R:]  -> [B, D, R]
    layer = base + einsum('bdr,bkr->bdk', x_a, x_b)
    out = x + einsum('bc,bco->bo', x, layer)

Key algebra: the B x D x D layer never needs to be materialized:
    out = x + x @ base + sum_r t[:, r] * x_b[:, :, r]
    t[b,r] = sum_d x[b,d] * x_a[b,d,r]

Distribution: RANK == n_cores == 8, so shard by rank r -- core i takes
the x_a and x_b columns of w2 belonging to rank i (stride-8 column
slices, 4096 of the 32768 columns = 1/8 of w2's 128MB). Each core then
computes, fully locally, with NO collectives:
  - h = gelu(ada_emb @ w1 + b1), replicated (w1 is 2MB bf16; measured on
    this runtime any collective costs 50-80us of launch-skew + cc-boot
    barrier, far more than the redundant compute),
  - t_i = sum over ALL d of x[:,d] * x_a[:,d,i]      (its own rank),
  - delta_i = t_i * x_b[:, :, i]                     [B, D], all of D,
  - (x + x @ base)[:, i*256:(i+1)*256]               (its base slice).
Core i's output is delta_i plus its base+residual slice: the output is
SUM-sharded and the host unshards by summing the 8 partials. Since SPMD
cores all run the identical program, the b-half columns are host-rotated
by -i*256 so each core's base slice lands at columns [0,256); the host
un-rotates with np.roll before summing.

All matmul operands are bf16 (1 cycle/row on the PE vs 4 for fp32, and
half the DMA bytes); PSUM accumulation and the t/delta arithmetic stay
fp32. Weight DMAs alternate between the sync and scalar HWDGE queues.
"""

import sys

import numpy as np

for _p in ("/opt/trn_rl_repo",):
    if _p not in sys.path:
        sys.path.insert(0, _p)

from concourse import bacc, bass, mybir, tile
from concourse.bass_utils import run_bass_kernel_spmd

B, D, ADA, INTER, RANK = 16, 2048, 1024, 1024, 8
NC = 8
DS = D // NC          # 256: per-core base/residual o-slice
KA = ADA // 128       # 8 k-tiles for the h matmul
KH = INTER // 128     # 8 k-tiles (and m-tiles) for h / xw
KX = D // 128         # 16 k-tiles for the base matmul
F32 = mybir.dt.float32
BF16 = mybir.dt.bfloat16

_CACHED_NC = None


def build_nc():
    nc = bacc.Bacc(
        "TRN2",
        target_bir_lowering=False,
        debug=False,
        enable_asserts=False,
        num_devices=NC,
    )

    ada_d = nc.declare_dram_parameter("ada", [128, KA * B], BF16, isOutput=False)
    w1_d = nc.declare_dram_parameter("w1f", [128, KA * INTER], BF16, isOutput=False)
    b1_d = nc.declare_dram_parameter("b1r", [1, INTER], BF16, isOutput=False)
    # per-core: rank-i columns of w2, a-half then b-half, k-tiled
    w2_d = nc.declare_dram_parameter("w2s", [128, KH * 2 * D], BF16, isOutput=False)
    b2_d = nc.declare_dram_parameter("b2s", [1, 2 * D], BF16, isOutput=False)
    xt_d = nc.declare_dram_parameter("xts", [128, KX * B], BF16, isOutput=False)
    ones_d = nc.declare_dram_parameter("ones", [1, B], BF16, isOutput=False)
    xs_d = nc.declare_dram_parameter("xs", [B, D], F32, isOutput=False)
    base_d = nc.declare_dram_parameter("bases", [128, KX * DS], BF16, isOutput=False)
    ident_d = nc.declare_dram_parameter("ident", [B, B], BF16, isOutput=False)
    xsb_d = nc.declare_dram_parameter("xsb", [B, DS], BF16, isOutput=False)
    identf_d = nc.declare_dram_parameter("identf", [B, B], F32, isOutput=False)
    out_d = nc.declare_dram_parameter("out", [B, D], F32, isOutput=True)

    with tile.TileContext(nc) as tc:
        with (
            tc.tile_pool(name="const", bufs=1) as cpool,
            tc.tile_pool(name="w2p", bufs=9) as w2pool,
            tc.tile_pool(name="ps", bufs=6, space="PSUM") as pp,
        ):
            # ---- small input loads (sync queue) ------------------------
            ada_sb = cpool.tile([128, KA * B], BF16)
            nc.sync.dma_start(ada_sb[:], ada_d[:])
            w1_sb = cpool.tile([128, KA * INTER], BF16)
            half = KA * INTER // 2
            nc.sync.dma_start(w1_sb[:, :half], w1_d[:, :half])
            nc.gpsimd.dma_start(w1_sb[:, half:], w1_d[:, half:])
            b1_sb = cpool.tile([1, INTER], BF16)
            nc.sync.dma_start(b1_sb[:], b1_d[:])
            xs_sb = cpool.tile([B, D], F32)
            nc.sync.dma_start(xs_sb[:], xs_d[:])
            b2_sb = cpool.tile([1, 2 * D], BF16)
            nc.sync.dma_start(b2_sb[:], b2_d[:])
            ones_sb = cpool.tile([1, B], BF16)
            nc.sync.dma_start(ones_sb[:], ones_d[:])
            xt_sb = cpool.tile([128, KX * B], BF16)
            nc.sync.dma_start(xt_sb[:], xt_d[:])
            ident_sb = cpool.tile([B, B], BF16)
            nc.sync.dma_start(ident_sb[:], ident_d[:])
            xsb_sb = cpool.tile([B, DS], BF16)
            nc.sync.dma_start(xsb_sb[:], xsb_d[:])
            identf_sb = cpool.tile([B, B], F32)
            nc.sync.dma_start(identf_sb[:], identf_d[:])

            # ---- h = gelu(ada @ w1 + b1), replicated -------------------
            # [16,1024] orientation: tiny [128,16] weight loads, N=512
            # streams; then PE-transpose into hT k-tiles for the xw stage.
            ph = pp.tile([B, 1024], F32, tag="ps2", name="hpre", bufs=1)
            for nn in range(2):
                for k in range(KA):
                    nc.tensor.matmul(
                        ph[:, nn * 512 : (nn + 1) * 512],
                        ada_sb[:, k * B : (k + 1) * B],
                        w1_sb[:, k * INTER + nn * 512 : k * INTER + (nn + 1) * 512],
                        start=(k == 0),
                        stop=False,
                    )
                nc.tensor.matmul(  # b1 bias via K=1 ones matmul
                    ph[:, nn * 512 : (nn + 1) * 512],
                    ones_sb[:],
                    b1_sb[0:1, nn * 512 : (nn + 1) * 512],
                    start=False,
                    stop=True,
                )
            h_sb = cpool.tile([B, INTER], F32)
            nc.scalar.activation(
                h_sb[:], ph[:], mybir.ActivationFunctionType.Gelu
            )
            ht_sb = cpool.tile([128, KH * B], BF16)
            for km in range(KH):
                pt = pp.tile([128, B], F32, tag="ps", name=f"pt{km}")
                nc.tensor.transpose(
                    pt[:], h_sb[:, km * 128 : (km + 1) * 128], identf_sb[:]
                )
                nc.scalar.activation(
                    ht_sb[:, km * B : (km + 1) * B], pt[:],
                    mybir.ActivationFunctionType.Copy,
                )

            # ---- base term + residual: x @ base[:, slice] + xs ---------
            base_ps = pp.tile([B, DS], F32, tag="ps", name="base_ps")
            bt = w2pool.tile([128, KX * DS], BF16, tag="w2", name="baset")
            nc.sync.dma_start(bt[:], base_d[:])
            for k in range(KX):
                nc.tensor.matmul(
                    base_ps[:],
                    xt_sb[:, k * B : (k + 1) * B],
                    bt[:, k * DS : (k + 1) * DS],
                    start=(k == 0),
                    stop=False,
                )
            nc.tensor.matmul(  # + residual: I^T @ xs == xs
                base_ps[:], ident_sb[:], xsb_sb[:], start=False, stop=True
            )

            # ---- xw a-half: n-major 1MB chunks, one PSUM bank each -----
            # chunk nb holds all 8 k-tiles for d-columns [nb*512,(nb+1)*512)
            psum_a = [pp.tile([B, 512], F32, tag="ps", name=f"psa{j}") for j in range(4)]
            psum_b = [pp.tile([B, 512], F32, tag="ps", name=f"psb{j}") for j in range(4)]
            tmp_t = cpool.tile([B, D], F32)
            t4 = cpool.tile([B, 4], F32)
            for nb in range(4):
                w2t = w2pool.tile([128, 2 * D], BF16, tag="w2", name=f"w2a{nb}")
                eng = nc.sync if nb % 2 == 0 else nc.gpsimd
                eng.dma_start(w2t[:], w2_d[:, nb * 4096 : (nb + 1) * 4096])
                for k in range(KH):
                    nc.tensor.matmul(
                        psum_a[nb][:],
                        ht_sb[:, k * B : (k + 1) * B],
                        w2t[:, k * 512 : (k + 1) * 512],
                        start=(k == 0),
                        stop=False,
                    )
                nc.tensor.matmul(  # b2 bias via K=1 ones matmul
                    psum_a[nb][:],
                    ones_sb[:],
                    b2_sb[0:1, nb * 512 : (nb + 1) * 512],
                    start=False,
                    stop=True,
                )
                # partial t for this bank, pipelined with the stream
                nc.vector.tensor_tensor(
                    tmp_t[:, nb * 512 : (nb + 1) * 512],
                    psum_a[nb][:],
                    xs_sb[:, nb * 512 : (nb + 1) * 512],
                    mybir.AluOpType.mult,
                )
                nc.vector.tensor_reduce(
                    t4[:, nb : nb + 1],
                    tmp_t[:, nb * 512 : (nb + 1) * 512],
                    axis=mybir.AxisListType.X, op=mybir.AluOpType.add,
                )
            t_sc = cpool.tile([B, 1], F32)
            nc.vector.tensor_reduce(
                t_sc[:], t4[:],
                axis=mybir.AxisListType.X, op=mybir.AluOpType.add,
            )

            # ---- xw b-half: n-major chunks, out muls pipelined ---------
            # last chunk split into two tiles so its matmuls overlap the
            # final 256KB of the stream (Tile deps are per-tile)
            out_sb = cpool.tile([B, D], F32)
            for nb in range(4):
                eng = nc.sync if nb % 2 == 0 else nc.gpsimd
                o = KH * D + nb * 4096
                if nb < 3:
                    w2t = w2pool.tile([128, 2 * D], BF16, tag="w2", name=f"w2b{nb}")
                    eng.dma_start(w2t[:], w2_d[:, o : o + 4096])
                    parts = [(w2t, 0, KH)]
                else:
                    w2t1 = w2pool.tile([128, 3072], BF16, tag="w2", name="w2b3a")
                    w2t2 = w2pool.tile([128, 1024], BF16, tag="w2tail", name="w2b3b")
                    eng.dma_start(w2t1[:], w2_d[:, o : o + 3072])
                    eng.dma_start(w2t2[:], w2_d[:, o + 3072 : o + 4096])
                    parts = [(w2t1, 0, 6), (w2t2, 6, KH)]
                for tile_, k0, k1 in parts:
                    for k in range(k0, k1):
                        nc.tensor.matmul(
                            psum_b[nb][:],
                            ht_sb[:, k * B : (k + 1) * B],
                            tile_[:, (k - k0) * 512 : (k - k0 + 1) * 512],
                            start=(k == 0),
                            stop=False,
                        )
                nc.tensor.matmul(
                    psum_b[nb][:],
                    ones_sb[:],
                    b2_sb[0:1, D + nb * 512 : D + (nb + 1) * 512],
                    start=False,
                    stop=True,
                )
                nc.vector.tensor_scalar_mul(
                    out_sb[:, nb * 512 : (nb + 1) * 512],
                    psum_b[nb][:],
                    t_sc[:, 0:1],
                )
                if nb == 0:  # base+residual lands in columns [0, DS)
                    nc.vector.tensor_tensor(
                        out_sb[:, 0:DS], out_sb[:, 0:DS], base_ps[:],
                        mybir.AluOpType.add,
                    )
                if nb == 2:  # stream out the finished 3/4 early
                    nc.sync.dma_start(out_d[:, 0:1536], out_sb[:, 0:1536])
            nc.sync.dma_start(out_d[:, 1536:], out_sb[:, 1536:])

    nc.compile()
    return nc


def _ktile(a: np.ndarray, p: int = 128) -> np.ndarray:
    """[K*p, m] -> [p, K*m] with free index = k*m + j (k-tile major)."""
    kp, m = a.shape
    k = kp // p
    return np.ascontiguousarray(
        a.reshape(k, p, m).transpose(1, 0, 2).reshape(p, k * m)
    )


def shard_inputs(x, ada_emb, base, w1, b1, w2, b2):
    import ml_dtypes

    bf16 = ml_dtypes.bfloat16
    x = np.ascontiguousarray(np.asarray(x, np.float32))
    ada_emb = np.asarray(ada_emb, np.float32)
    base = np.asarray(base, np.float32)
    w1 = np.asarray(w1, np.float32)
    b1 = np.asarray(b1, np.float32)
    w2 = np.asarray(w2, bf16)
    b2 = np.asarray(b2, np.float32)

    ada_pre = _ktile(np.ascontiguousarray(ada_emb.T)).astype(bf16)  # [128, 8*16]
    xt_pre = _ktile(np.ascontiguousarray(x.T)).astype(bf16)         # [128, 16*16]
    w1f = _ktile(w1).astype(bf16)                                  # [128, 8*1024]
    b1r = b1.reshape(1, INTER).astype(bf16)

    w2a, w2b = w2[:, : D * RANK], w2[:, D * RANK :]
    b2a, b2b = b2[: D * RANK], b2[D * RANK :]
    in_maps = []
    for i in range(NC):
        # rank-i columns: stride-RANK slices; b-half rotated by -i*DS so
        # the base/residual slice lands at output columns [0, DS)
        # n-major blocks: free idx = nb*(8*512) + k*512 + c
        def _nmajor(a):
            return np.ascontiguousarray(
                a.reshape(KH, 128, 4, 512).transpose(1, 2, 0, 3).reshape(128, KH * 2048)
            )
        w2ai = _nmajor(np.ascontiguousarray(w2a[:, i::RANK]))
        w2bi = _nmajor(np.ascontiguousarray(np.roll(w2b[:, i::RANK], -i * DS, axis=1)))
        in_maps.append({
            "ada": ada_pre,
            "w1f": w1f,
            "b1r": b1r,
            "w2s": np.ascontiguousarray(np.concatenate([w2ai, w2bi], axis=1)),
            "b2s": np.concatenate(
                [b2a[i::RANK], np.roll(b2b[i::RANK], -i * DS)]
            ).reshape(1, -1).astype(bf16),
            "ones": np.ones((1, B), bf16),
            "xts": xt_pre,
            "xs": x,
            "ident": np.eye(B, dtype=bf16),
            "identf": np.eye(B, dtype=np.float32),
            "xsb": np.ascontiguousarray(x[:, i * DS : (i + 1) * DS]).astype(bf16),
            "bases": _ktile(base[:, i * DS : (i + 1) * DS]).astype(bf16),
        })
    return in_maps


def kernel(**inputs) -> np.ndarray:
    global _CACHED_NC
    if _CACHED_NC is None:
        _CACHED_NC = build_nc()
    in_maps = shard_inputs(**inputs)
    res = run_bass_kernel_spmd(_CACHED_NC, in_maps, list(range(NC)))
    # Each core's "out" is a sum-shard of the output, column-rotated by
    # -i*DS. Un-rotate and sum to unshard.
    total = np.zeros((B, D), np.float32)
    for i in range(NC):
        total += np.roll(res.results[i]["out"], i * DS, axis=1)
    return total


if __name__ == "__main__":
    rng = np.random.default_rng(0)
    ins = {
        "x": rng.standard_normal((B, D), np.float32),
        "ada_emb": rng.standard_normal((B, ADA), np.float32),
        "base": rng.standard_normal((D, D), np.float32),
        "w1": rng.standard_normal((ADA, INTER), np.float32) / np.sqrt(ADA),
        "b1": rng.standard_normal((INTER,), np.float32) / np.sqrt(ADA),
        "w2": rng.standard_normal((INTER, D * RANK * 2), np.float32) / np.sqrt(INTER),
        "b2": rng.standard_normal((D * RANK * 2,), np.float32) / np.sqrt(INTER),
    }
    out = kernel(**ins)
    print("out", out.shape, out.dtype, float(np.abs(out).mean()))


# revision 34
# speedup vs baseline: 1.1204x; 1.1204x over previous
"""AdaLoRAWithBase distributed Trainium2 kernel (8 NeuronCores).

Reference computation (B=16, D=2048, ADA=1024, INTER=1024, R=8):
    h   = gelu(ada_emb @ w1 + b1)                  [B, INTER]
    xw  = h @ w2 + b2                              [B, 2*D*R]
    x_a = xw[:, :D*R]  -> [B, D, R]
    x_b = xw[:, D*R:]  -> [B, D, R]
    layer = base + einsum('bdr,bkr->bdk', x_a, x_b)
    out = x + einsum('bc,bco->bo', x, layer)

Key algebra: the B x D x D layer never needs to be materialized:
    out = x + x @ base + sum_r t[:, r] * x_b[:, :, r]
    t[b,r] = sum_d x[b,d] * x_a[b,d,r]

Distribution: RANK == n_cores == 8, so shard by rank r -- core i takes
the x_a and x_b columns of w2 belonging to rank i (stride-8 column
slices, 4096 of the 32768 columns = 1/8 of w2's 128MB). Each core then
computes, fully locally, with NO collectives:
  - h = gelu(ada_emb @ w1 + b1), replicated (w1 is 2MB bf16; measured on
    this runtime any collective costs 50-80us of launch-skew + cc-boot
    barrier, far more than the redundant compute),
  - t_i = sum over ALL d of x[:,d] * x_a[:,d,i]      (its own rank),
  - delta_i = t_i * x_b[:, :, i]                     [B, D], all of D,
  - (x + x @ base)[:, i*256:(i+1)*256]               (its base slice).
Core i's output is delta_i plus its base+residual slice: the output is
SUM-sharded and the host unshards by summing the 8 partials. Since SPMD
cores all run the identical program, the b-half columns are host-rotated
by -i*256 so each core's base slice lands at columns [0,256); the host
un-rotates with np.roll before summing.

All matmul operands are bf16 (1 cycle/row on the PE vs 4 for fp32, and
half the DMA bytes); PSUM accumulation and the t/delta arithmetic stay
fp32. Weight DMAs alternate between the sync and scalar HWDGE queues.
"""

import sys

import numpy as np

for _p in ("/opt/trn_rl_repo",):
    if _p not in sys.path:
        sys.path.insert(0, _p)

from concourse import bacc, bass, mybir, tile
from concourse.bass_utils import run_bass_kernel_spmd

B, D, ADA, INTER, RANK = 16, 2048, 1024, 1024, 8
NC = 8
DS = D // NC          # 256: per-core base/residual o-slice
KA = ADA // 128       # 8 k-tiles for the h matmul
KH = INTER // 128     # 8 k-tiles (and m-tiles) for h / xw
KX = D // 128         # 16 k-tiles for the base matmul
F32 = mybir.dt.float32
BF16 = mybir.dt.bfloat16

_CACHED_NC = None


def build_nc():
    nc = bacc.Bacc(
        "TRN2",
        target_bir_lowering=False,
        debug=False,
        enable_asserts=False,
        num_devices=NC,
    )

    ada_d = nc.declare_dram_parameter("ada", [128, KA * B], BF16, isOutput=False)
    w1_d = nc.declare_dram_parameter("w1f", [128, KA * INTER], BF16, isOutput=False)
    b1_d = nc.declare_dram_parameter("b1r", [1, INTER], BF16, isOutput=False)
    # per-core: rank-i columns of w2, a-half then b-half, k-tiled
    w2_d = nc.declare_dram_parameter("w2s", [128, KH * 2 * D], BF16, isOutput=False)
    b2_d = nc.declare_dram_parameter("b2s", [1, 2 * D], BF16, isOutput=False)
    xt_d = nc.declare_dram_parameter("xts", [128, KX * B], BF16, isOutput=False)
    ones_d = nc.declare_dram_parameter("ones", [1, B], BF16, isOutput=False)
    xs_d = nc.declare_dram_parameter("xs", [B, D], F32, isOutput=False)
    base_d = nc.declare_dram_parameter("bases", [128, KX * DS], BF16, isOutput=False)
    ident_d = nc.declare_dram_parameter("ident", [B, B], BF16, isOutput=False)
    xsb_d = nc.declare_dram_parameter("xsb", [B, DS], BF16, isOutput=False)
    identf_d = nc.declare_dram_parameter("identf", [B, B], F32, isOutput=False)
    out_d = nc.declare_dram_parameter("out", [B, D], F32, isOutput=True)

    with tile.TileContext(nc) as tc:
        with (
            tc.tile_pool(name="const", bufs=1) as cpool,
            tc.tile_pool(name="w2p", bufs=6) as w2pool,
            tc.tile_pool(name="ps", bufs=6, space="PSUM") as pp,
        ):
            # ---- small input loads (sync queue) ------------------------
            ada_sb = cpool.tile([128, KA * B], BF16)
            nc.sync.dma_start(ada_sb[:], ada_d[:])
            w1_sb = cpool.tile([128, KA * INTER], BF16)
            half = KA * INTER // 2
            nc.sync.dma_start(w1_sb[:, :half], w1_d[:, :half])
            nc.gpsimd.dma_start(w1_sb[:, half:], w1_d[:, half:])
            b1_sb = cpool.tile([1, INTER], BF16)
            nc.sync.dma_start(b1_sb[:], b1_d[:])
            xs_sb = cpool.tile([B, D], F32)
            nc.sync.dma_start(xs_sb[:], xs_d[:])
            b2_sb = cpool.tile([1, 2 * D], BF16)
            nc.sync.dma_start(b2_sb[:], b2_d[:])
            ones_sb = cpool.tile([1, B], BF16)
            nc.sync.dma_start(ones_sb[:], ones_d[:])
            xt_sb = cpool.tile([128, KX * B], BF16)
            nc.sync.dma_start(xt_sb[:], xt_d[:])
            ident_sb = cpool.tile([B, B], BF16)
            nc.sync.dma_start(ident_sb[:], ident_d[:])
            xsb_sb = cpool.tile([B, DS], BF16)
            nc.sync.dma_start(xsb_sb[:], xsb_d[:])
            identf_sb = cpool.tile([B, B], F32)
            nc.sync.dma_start(identf_sb[:], identf_d[:])

            # ---- h = gelu(ada @ w1 + b1), replicated -------------------
            # [16,1024] orientation: tiny [128,16] weight loads, N=512
            # streams; then PE-transpose into hT k-tiles for the xw stage.
            ph = pp.tile([B, 1024], F32, tag="ps2", name="hpre", bufs=1)
            for nn in range(2):
                for k in range(KA):
                    nc.tensor.matmul(
                        ph[:, nn * 512 : (nn + 1) * 512],
                        ada_sb[:, k * B : (k + 1) * B],
                        w1_sb[:, k * INTER + nn * 512 : k * INTER + (nn + 1) * 512],
                        start=(k == 0),
                        stop=False,
                    )
                nc.tensor.matmul(  # b1 bias via K=1 ones matmul
                    ph[:, nn * 512 : (nn + 1) * 512],
                    ones_sb[:],
                    b1_sb[0:1, nn * 512 : (nn + 1) * 512],
                    start=False,
                    stop=True,
                )
            h_sb = cpool.tile([B, INTER], F32)
            nc.scalar.activation(
                h_sb[:], ph[:], mybir.ActivationFunctionType.Gelu
            )
            ht_sb = cpool.tile([128, KH * B], BF16)
            for km in range(KH):
                pt = pp.tile([128, B], F32, tag="ps", name=f"pt{km}")
                nc.tensor.transpose(
                    pt[:], h_sb[:, km * 128 : (km + 1) * 128], identf_sb[:]
                )
                nc.scalar.activation(
                    ht_sb[:, km * B : (km + 1) * B], pt[:],
                    mybir.ActivationFunctionType.Copy,
                )

            # ---- base term + residual: x @ base[:, slice] + xs ---------
            base_ps = pp.tile([B, DS], F32, tag="ps", name="base_ps")
            bt = w2pool.tile([128, KX * DS], BF16, tag="w2", name="baset")
            nc.sync.dma_start(bt[:], base_d[:])
            for k in range(KX):
                nc.tensor.matmul(
                    base_ps[:],
                    xt_sb[:, k * B : (k + 1) * B],
                    bt[:, k * DS : (k + 1) * DS],
                    start=(k == 0),
                    stop=False,
                )
            nc.tensor.matmul(  # + residual: I^T @ xs == xs
                base_ps[:], ident_sb[:], xsb_sb[:], start=False, stop=True
            )

            # ---- xw a-half: n-major 1MB chunks, one PSUM bank each -----
            # chunk nb holds all 8 k-tiles for d-columns [nb*512,(nb+1)*512)
            psum_a = [pp.tile([B, 512], F32, tag="ps", name=f"psa{j}") for j in range(4)]
            psum_b = [pp.tile([B, 512], F32, tag="ps", name=f"psb{j}") for j in range(4)]
            tmp_t = cpool.tile([B, D], F32)
            t4 = cpool.tile([B, 4], F32)
            for nb in range(4):
                w2t = w2pool.tile([128, 2 * D], BF16, tag="w2", name=f"w2a{nb}")
                eng = nc.sync if nb % 2 == 0 else nc.gpsimd
                eng.dma_start(w2t[:], w2_d[:, nb * 4096 : (nb + 1) * 4096])
                for k in range(KH):
                    nc.tensor.matmul(
                        psum_a[nb][:],
                        ht_sb[:, k * B : (k + 1) * B],
                        w2t[:, k * 512 : (k + 1) * 512],
                        start=(k == 0),
                        stop=False,
                    )
                nc.tensor.matmul(  # b2 bias via K=1 ones matmul
                    psum_a[nb][:],
                    ones_sb[:],
                    b2_sb[0:1, nb * 512 : (nb + 1) * 512],
                    start=False,
                    stop=True,
                )
                # partial t for this bank, pipelined with the stream
                nc.vector.tensor_tensor(
                    tmp_t[:, nb * 512 : (nb + 1) * 512],
                    psum_a[nb][:],
                    xs_sb[:, nb * 512 : (nb + 1) * 512],
                    mybir.AluOpType.mult,
                )
                nc.vector.tensor_reduce(
                    t4[:, nb : nb + 1],
                    tmp_t[:, nb * 512 : (nb + 1) * 512],
                    axis=mybir.AxisListType.X, op=mybir.AluOpType.add,
                )
            t_sc = cpool.tile([B, 1], F32)
            nc.vector.tensor_reduce(
                t_sc[:], t4[:],
                axis=mybir.AxisListType.X, op=mybir.AluOpType.add,
            )

            # ---- xw b-half: n-major chunks, out muls pipelined ---------
            # last chunk split into two tiles so its matmuls overlap the
            # final 256KB of the stream (Tile deps are per-tile)
            out_sb = cpool.tile([B, D], F32)
            for nb in range(4):
                eng = nc.sync if nb % 2 == 0 else nc.gpsimd
                o = KH * D + nb * 4096
                if nb < 3:
                    w2t = w2pool.tile([128, 2 * D], BF16, tag="w2", name=f"w2b{nb}")
                    eng.dma_start(w2t[:], w2_d[:, o : o + 4096])
                    parts = [(w2t, 0, KH)]
                else:
                    w2t1 = w2pool.tile([128, 3072], BF16, tag="w2", name="w2b3a")
                    w2t2 = w2pool.tile([128, 1024], BF16, tag="w2tail", name="w2b3b")
                    eng.dma_start(w2t1[:], w2_d[:, o : o + 3072])
                    eng.dma_start(w2t2[:], w2_d[:, o + 3072 : o + 4096])
                    parts = [(w2t1, 0, 6), (w2t2, 6, KH)]
                for tile_, k0, k1 in parts:
                    for k in range(k0, k1):
                        nc.tensor.matmul(
                            psum_b[nb][:],
                            ht_sb[:, k * B : (k + 1) * B],
                            tile_[:, (k - k0) * 512 : (k - k0 + 1) * 512],
                            start=(k == 0),
                            stop=False,
                        )
                nc.tensor.matmul(
                    psum_b[nb][:],
                    ones_sb[:],
                    b2_sb[0:1, D + nb * 512 : D + (nb + 1) * 512],
                    start=False,
                    stop=True,
                )
                nc.vector.tensor_scalar_mul(
                    out_sb[:, nb * 512 : (nb + 1) * 512],
                    psum_b[nb][:],
                    t_sc[:, 0:1],
                )
                if nb == 0:  # base+residual lands in columns [0, DS)
                    nc.vector.tensor_tensor(
                        out_sb[:, 0:DS], out_sb[:, 0:DS], base_ps[:],
                        mybir.AluOpType.add,
                    )
                if nb == 2:  # stream out the finished 3/4 early
                    nc.sync.dma_start(out_d[:, 0:1536], out_sb[:, 0:1536])
            nc.sync.dma_start(out_d[:, 1536:], out_sb[:, 1536:])

    nc.compile()
    return nc


def _ktile(a: np.ndarray, p: int = 128) -> np.ndarray:
    """[K*p, m] -> [p, K*m] with free index = k*m + j (k-tile major)."""
    kp, m = a.shape
    k = kp // p
    return np.ascontiguousarray(
        a.reshape(k, p, m).transpose(1, 0, 2).reshape(p, k * m)
    )


def shard_inputs(x, ada_emb, base, w1, b1, w2, b2):
    import ml_dtypes

    bf16 = ml_dtypes.bfloat16
    x = np.ascontiguousarray(np.asarray(x, np.float32))
    ada_emb = np.asarray(ada_emb, np.float32)
    base = np.asarray(base, np.float32)
    w1 = np.asarray(w1, np.float32)
    b1 = np.asarray(b1, np.float32)
    w2 = np.asarray(w2, bf16)
    b2 = np.asarray(b2, np.float32)

    ada_pre = _ktile(np.ascontiguousarray(ada_emb.T)).astype(bf16)  # [128, 8*16]
    xt_pre = _ktile(np.ascontiguousarray(x.T)).astype(bf16)         # [128, 16*16]
    w1f = _ktile(w1).astype(bf16)                                  # [128, 8*1024]
    b1r = b1.reshape(1, INTER).astype(bf16)

    w2a, w2b = w2[:, : D * RANK], w2[:, D * RANK :]
    b2a, b2b = b2[: D * RANK], b2[D * RANK :]
    in_maps = []
    for i in range(NC):
        # rank-i columns: stride-RANK slices; b-half rotated by -i*DS so
        # the base/residual slice lands at output columns [0, DS)
        # n-major blocks: free idx = nb*(8*512) + k*512 + c
        def _nmajor(a):
            return np.ascontiguousarray(
                a.reshape(KH, 128, 4, 512).transpose(1, 2, 0, 3).reshape(128, KH * 2048)
            )
        w2ai = _nmajor(np.ascontiguousarray(w2a[:, i::RANK]))
        w2bi = _nmajor(np.ascontiguousarray(np.roll(w2b[:, i::RANK], -i * DS, axis=1)))
        in_maps.append({
            "ada": ada_pre,
            "w1f": w1f,
            "b1r": b1r,
            "w2s": np.ascontiguousarray(np.concatenate([w2ai, w2bi], axis=1)),
            "b2s": np.concatenate(
                [b2a[i::RANK], np.roll(b2b[i::RANK], -i * DS)]
            ).reshape(1, -1).astype(bf16),
            "ones": np.ones((1, B), bf16),
            "xts": xt_pre,
            "xs": x,
            "ident": np.eye(B, dtype=bf16),
            "identf": np.eye(B, dtype=np.float32),
            "xsb": np.ascontiguousarray(x[:, i * DS : (i + 1) * DS]).astype(bf16),
            "bases": _ktile(base[:, i * DS : (i + 1) * DS]).astype(bf16),
        })
    return in_maps


def kernel(**inputs) -> np.ndarray:
    global _CACHED_NC
    if _CACHED_NC is None:
        _CACHED_NC = build_nc()
    in_maps = shard_inputs(**inputs)
    res = run_bass_kernel_spmd(_CACHED_NC, in_maps, list(range(NC)))
    # Each core's "out" is a sum-shard of the output, column-rotated by
    # -i*DS. Un-rotate and sum to unshard.
    total = np.zeros((B, D), np.float32)
    for i in range(NC):
        total += np.roll(res.results[i]["out"], i * DS, axis=1)
    return total


if __name__ == "__main__":
    rng = np.random.default_rng(0)
    ins = {
        "x": rng.standard_normal((B, D), np.float32),
        "ada_emb": rng.standard_normal((B, ADA), np.float32),
        "base": rng.standard_normal((D, D), np.float32),
        "w1": rng.standard_normal((ADA, INTER), np.float32) / np.sqrt(ADA),
        "b1": rng.standard_normal((INTER,), np.float32) / np.sqrt(ADA),
        "w2": rng.standard_normal((INTER, D * RANK * 2), np.float32) / np.sqrt(INTER),
        "b2": rng.standard_normal((D * RANK * 2,), np.float32) / np.sqrt(INTER),
    }
    out = kernel(**ins)
    print("out", out.shape, out.dtype, float(np.abs(out).mean()))


# revision 35
# speedup vs baseline: 1.1423x; 1.0195x over previous
"""AdaLoRAWithBase distributed Trainium2 kernel (8 NeuronCores).

Reference computation (B=16, D=2048, ADA=1024, INTER=1024, R=8):
    h   = gelu(ada_emb @ w1 + b1)                  [B, INTER]
    xw  = h @ w2 + b2                              [B, 2*D*R]
    x_a = xw[:, :D*R]  -> [B, D, R]
    x_b = xw[:, D*R:]  -> [B, D, R]
    layer = base + einsum('bdr,bkr->bdk', x_a, x_b)
    out = x + einsum('bc,bco->bo', x, layer)

Key algebra: the B x D x D layer never needs to be materialized:
    out = x + x @ base + sum_r t[:, r] * x_b[:, :, r]
    t[b,r] = sum_d x[b,d] * x_a[b,d,r]

Distribution: RANK == n_cores == 8, so shard by rank r -- core i takes
the x_a and x_b columns of w2 belonging to rank i (stride-8 column
slices, 4096 of the 32768 columns = 1/8 of w2's 128MB). Each core then
computes, fully locally, with NO collectives:
  - h = gelu(ada_emb @ w1 + b1), replicated (w1 is 2MB bf16; measured on
    this runtime any collective costs 50-80us of launch-skew + cc-boot
    barrier, far more than the redundant compute),
  - t_i = sum over ALL d of x[:,d] * x_a[:,d,i]      (its own rank),
  - delta_i = t_i * x_b[:, :, i]                     [B, D], all of D,
  - (x + x @ base)[:, i*256:(i+1)*256]               (its base slice).
Core i's output is delta_i plus its base+residual slice: the output is
SUM-sharded and the host unshards by summing the 8 partials. Since SPMD
cores all run the identical program, the b-half columns are host-rotated
by -i*256 so each core's base slice lands at columns [0,256); the host
un-rotates with np.roll before summing.

All matmul operands are bf16 (1 cycle/row on the PE vs 4 for fp32, and
half the DMA bytes); PSUM accumulation and the t/delta arithmetic stay
fp32. Weight DMAs alternate between the sync and scalar HWDGE queues.
"""

import sys

import numpy as np

for _p in ("/opt/trn_rl_repo",):
    if _p not in sys.path:
        sys.path.insert(0, _p)

from concourse import bacc, bass, mybir, tile
from concourse.bass_utils import run_bass_kernel_spmd

B, D, ADA, INTER, RANK = 16, 2048, 1024, 1024, 8
NC = 8
DS = D // NC          # 256: per-core base/residual o-slice
KA = ADA // 128       # 8 k-tiles for the h matmul
KH = INTER // 128     # 8 k-tiles (and m-tiles) for h / xw
KX = D // 128         # 16 k-tiles for the base matmul
F32 = mybir.dt.float32
BF16 = mybir.dt.bfloat16

_CACHED_NC = None


def build_nc():
    nc = bacc.Bacc(
        "TRN2",
        target_bir_lowering=False,
        debug=False,
        enable_asserts=False,
        num_devices=NC,
    )

    ada_d = nc.declare_dram_parameter("ada", [128, KA * B], BF16, isOutput=False)
    w1_d = nc.declare_dram_parameter("w1f", [128, KA * INTER], BF16, isOutput=False)
    b1_d = nc.declare_dram_parameter("b1r", [1, INTER], BF16, isOutput=False)
    # per-core: rank-i columns of w2, a-half then b-half, k-tiled
    w2_d = nc.declare_dram_parameter("w2s", [128, KH * 2 * D], BF16, isOutput=False)
    b2_d = nc.declare_dram_parameter("b2s", [1, 2 * D], BF16, isOutput=False)
    xt_d = nc.declare_dram_parameter("xts", [128, KX * B], BF16, isOutput=False)
    ones_d = nc.declare_dram_parameter("ones", [1, B], BF16, isOutput=False)
    xs_d = nc.declare_dram_parameter("xs", [B, D], F32, isOutput=False)
    base_d = nc.declare_dram_parameter("bases", [128, KX * DS], BF16, isOutput=False)
    ident_d = nc.declare_dram_parameter("ident", [B, B], BF16, isOutput=False)
    xsb_d = nc.declare_dram_parameter("xsb", [B, DS], BF16, isOutput=False)
    identf_d = nc.declare_dram_parameter("identf", [B, B], F32, isOutput=False)
    out_d = nc.declare_dram_parameter("out", [B, D], F32, isOutput=True)

    with tile.TileContext(nc) as tc:
        with (
            tc.tile_pool(name="const", bufs=1) as cpool,
            tc.tile_pool(name="w2p", bufs=6) as w2pool,
            tc.tile_pool(name="ps", bufs=6, space="PSUM") as pp,
        ):
            # ---- small input loads (sync queue) ------------------------
            ada_sb = cpool.tile([128, KA * B], BF16)
            nc.sync.dma_start(ada_sb[:], ada_d[:])
            w1_sb = cpool.tile([128, KA * INTER], BF16)
            half = KA * INTER // 2
            nc.sync.dma_start(w1_sb[:, :half], w1_d[:, :half])
            nc.gpsimd.dma_start(w1_sb[:, half:], w1_d[:, half:])
            b1_sb = cpool.tile([1, INTER], BF16)
            nc.sync.dma_start(b1_sb[:], b1_d[:])
            xs_sb = cpool.tile([B, D], F32)
            nc.sync.dma_start(xs_sb[:], xs_d[:])
            b2_sb = cpool.tile([1, 2 * D], BF16)
            nc.sync.dma_start(b2_sb[:], b2_d[:])
            ones_sb = cpool.tile([1, B], BF16)
            nc.sync.dma_start(ones_sb[:], ones_d[:])
            xt_sb = cpool.tile([128, KX * B], BF16)
            nc.sync.dma_start(xt_sb[:], xt_d[:])
            ident_sb = cpool.tile([B, B], BF16)
            nc.sync.dma_start(ident_sb[:], ident_d[:])
            xsb_sb = cpool.tile([B, DS], BF16)
            nc.sync.dma_start(xsb_sb[:], xsb_d[:])
            identf_sb = cpool.tile([B, B], F32)
            nc.sync.dma_start(identf_sb[:], identf_d[:])

            # ---- h = gelu(ada @ w1 + b1), replicated -------------------
            # [16,1024] orientation: tiny [128,16] weight loads, N=512
            # streams; then PE-transpose into hT k-tiles for the xw stage.
            ph = pp.tile([B, 1024], F32, tag="ps2", name="hpre", bufs=1)
            for nn in range(2):
                for k in range(KA):
                    nc.tensor.matmul(
                        ph[:, nn * 512 : (nn + 1) * 512],
                        ada_sb[:, k * B : (k + 1) * B],
                        w1_sb[:, k * INTER + nn * 512 : k * INTER + (nn + 1) * 512],
                        start=(k == 0),
                        stop=False,
                    )
                nc.tensor.matmul(  # b1 bias via K=1 ones matmul
                    ph[:, nn * 512 : (nn + 1) * 512],
                    ones_sb[:],
                    b1_sb[0:1, nn * 512 : (nn + 1) * 512],
                    start=False,
                    stop=True,
                )
            h_sb = cpool.tile([B, INTER], F32)
            ht_sb = cpool.tile([128, KH * B], BF16)
            for nn in range(2):  # per-half gelu+transpose: k<4 tiles ready early
                nc.scalar.activation(
                    h_sb[:, nn * 512 : (nn + 1) * 512],
                    ph[:, nn * 512 : (nn + 1) * 512],
                    mybir.ActivationFunctionType.Gelu,
                )
                for km in range(nn * 4, nn * 4 + 4):
                    pt = pp.tile([128, B], F32, tag="ps", name=f"pt{km}")
                    nc.tensor.transpose(
                        pt[:], h_sb[:, km * 128 : (km + 1) * 128], identf_sb[:]
                    )
                    nc.scalar.activation(
                        ht_sb[:, km * B : (km + 1) * B], pt[:],
                        mybir.ActivationFunctionType.Copy,
                    )

            # ---- base term + residual: x @ base[:, slice] + xs ---------
            base_ps = pp.tile([B, DS], F32, tag="ps", name="base_ps")
            bt = w2pool.tile([128, KX * DS], BF16, tag="w2", name="baset")
            nc.sync.dma_start(bt[:], base_d[:])
            for k in range(KX):
                nc.tensor.matmul(
                    base_ps[:],
                    xt_sb[:, k * B : (k + 1) * B],
                    bt[:, k * DS : (k + 1) * DS],
                    start=(k == 0),
                    stop=False,
                )
            nc.tensor.matmul(  # + residual: I^T @ xs == xs
                base_ps[:], ident_sb[:], xsb_sb[:], start=False, stop=True
            )

            # ---- xw a-half: n-major 1MB chunks, one PSUM bank each -----
            # chunk nb holds all 8 k-tiles for d-columns [nb*512,(nb+1)*512)
            psum_a = [pp.tile([B, 512], F32, tag="ps", name=f"psa{j}") for j in range(4)]
            psum_b = [pp.tile([B, 512], F32, tag="ps", name=f"psb{j}") for j in range(4)]
            tmp_t = cpool.tile([B, D], F32)
            t4 = cpool.tile([B, 4], F32)
            for nb in range(4):
                w2t = w2pool.tile([128, 2 * D], BF16, tag="w2", name=f"w2a{nb}")
                eng = nc.sync if nb % 2 == 0 else nc.gpsimd
                eng.dma_start(w2t[:], w2_d[:, nb * 4096 : (nb + 1) * 4096])
                for k in range(KH):
                    nc.tensor.matmul(
                        psum_a[nb][:],
                        ht_sb[:, k * B : (k + 1) * B],
                        w2t[:, k * 512 : (k + 1) * 512],
                        start=(k == 0),
                        stop=False,
                    )
                nc.tensor.matmul(  # b2 bias via K=1 ones matmul
                    psum_a[nb][:],
                    ones_sb[:],
                    b2_sb[0:1, nb * 512 : (nb + 1) * 512],
                    start=False,
                    stop=True,
                )
                # partial t for this bank, pipelined with the stream
                nc.vector.tensor_tensor(
                    tmp_t[:, nb * 512 : (nb + 1) * 512],
                    psum_a[nb][:],
                    xs_sb[:, nb * 512 : (nb + 1) * 512],
                    mybir.AluOpType.mult,
                )
                nc.vector.tensor_reduce(
                    t4[:, nb : nb + 1],
                    tmp_t[:, nb * 512 : (nb + 1) * 512],
                    axis=mybir.AxisListType.X, op=mybir.AluOpType.add,
                )
            t_sc = cpool.tile([B, 1], F32)
            nc.vector.tensor_reduce(
                t_sc[:], t4[:],
                axis=mybir.AxisListType.X, op=mybir.AluOpType.add,
            )

            # ---- xw b-half: n-major chunks, out muls pipelined ---------
            # last chunk split into two tiles so its matmuls overlap the
            # final 256KB of the stream (Tile deps are per-tile)
            out_sb = cpool.tile([B, D], F32)
            for nb in range(4):
                eng = nc.sync if nb % 2 == 0 else nc.gpsimd
                o = KH * D + nb * 4096
                if nb < 3:
                    w2t = w2pool.tile([128, 2 * D], BF16, tag="w2", name=f"w2b{nb}")
                    eng.dma_start(w2t[:], w2_d[:, o : o + 4096])
                    parts = [(w2t, 0, KH)]
                else:
                    w2t1 = w2pool.tile([128, 3072], BF16, tag="w2", name="w2b3a")
                    w2t2 = w2pool.tile([128, 1024], BF16, tag="w2tail", name="w2b3b")
                    eng.dma_start(w2t1[:], w2_d[:, o : o + 3072])
                    eng.dma_start(w2t2[:], w2_d[:, o + 3072 : o + 4096])
                    parts = [(w2t1, 0, 6), (w2t2, 6, KH)]
                for tile_, k0, k1 in parts:
                    for k in range(k0, k1):
                        nc.tensor.matmul(
                            psum_b[nb][:],
                            ht_sb[:, k * B : (k + 1) * B],
                            tile_[:, (k - k0) * 512 : (k - k0 + 1) * 512],
                            start=(k == 0),
                            stop=False,
                        )
                nc.tensor.matmul(
                    psum_b[nb][:],
                    ones_sb[:],
                    b2_sb[0:1, D + nb * 512 : D + (nb + 1) * 512],
                    start=False,
                    stop=True,
                )
                nc.vector.tensor_scalar_mul(
                    out_sb[:, nb * 512 : (nb + 1) * 512],
                    psum_b[nb][:],
                    t_sc[:, 0:1],
                )
                if nb == 0:  # base+residual lands in columns [0, DS)
                    nc.vector.tensor_tensor(
                        out_sb[:, 0:DS], out_sb[:, 0:DS], base_ps[:],
                        mybir.AluOpType.add,
                    )
                if nb == 2:  # stream out the finished 3/4 early
                    nc.sync.dma_start(out_d[:, 0:1536], out_sb[:, 0:1536])
            nc.sync.dma_start(out_d[:, 1536:], out_sb[:, 1536:])

    nc.compile()
    return nc


def _ktile(a: np.ndarray, p: int = 128) -> np.ndarray:
    """[K*p, m] -> [p, K*m] with free index = k*m + j (k-tile major)."""
    kp, m = a.shape
    k = kp // p
    return np.ascontiguousarray(
        a.reshape(k, p, m).transpose(1, 0, 2).reshape(p, k * m)
    )


def shard_inputs(x, ada_emb, base, w1, b1, w2, b2):
    import ml_dtypes

    bf16 = ml_dtypes.bfloat16
    x = np.ascontiguousarray(np.asarray(x, np.float32))
    ada_emb = np.asarray(ada_emb, np.float32)
    base = np.asarray(base, np.float32)
    w1 = np.asarray(w1, np.float32)
    b1 = np.asarray(b1, np.float32)
    w2 = np.asarray(w2, bf16)
    b2 = np.asarray(b2, np.float32)

    ada_pre = _ktile(np.ascontiguousarray(ada_emb.T)).astype(bf16)  # [128, 8*16]
    xt_pre = _ktile(np.ascontiguousarray(x.T)).astype(bf16)         # [128, 16*16]
    w1f = _ktile(w1).astype(bf16)                                  # [128, 8*1024]
    b1r = b1.reshape(1, INTER).astype(bf16)

    w2a, w2b = w2[:, : D * RANK], w2[:, D * RANK :]
    b2a, b2b = b2[: D * RANK], b2[D * RANK :]
    in_maps = []
    for i in range(NC):
        # rank-i columns: stride-RANK slices; b-half rotated by -i*DS so
        # the base/residual slice lands at output columns [0, DS)
        # n-major blocks: free idx = nb*(8*512) + k*512 + c
        def _nmajor(a):
            return np.ascontiguousarray(
                a.reshape(KH, 128, 4, 512).transpose(1, 2, 0, 3).reshape(128, KH * 2048)
            )
        w2ai = _nmajor(np.ascontiguousarray(w2a[:, i::RANK]))
        w2bi = _nmajor(np.ascontiguousarray(np.roll(w2b[:, i::RANK], -i * DS, axis=1)))
        in_maps.append({
            "ada": ada_pre,
            "w1f": w1f,
            "b1r": b1r,
            "w2s": np.ascontiguousarray(np.concatenate([w2ai, w2bi], axis=1)),
            "b2s": np.concatenate(
                [b2a[i::RANK], np.roll(b2b[i::RANK], -i * DS)]
            ).reshape(1, -1).astype(bf16),
            "ones": np.ones((1, B), bf16),
            "xts": xt_pre,
            "xs": x,
            "ident": np.eye(B, dtype=bf16),
            "identf": np.eye(B, dtype=np.float32),
            "xsb": np.ascontiguousarray(x[:, i * DS : (i + 1) * DS]).astype(bf16),
            "bases": _ktile(base[:, i * DS : (i + 1) * DS]).astype(bf16),
        })
    return in_maps


def kernel(**inputs) -> np.ndarray:
    global _CACHED_NC
    if _CACHED_NC is None:
        _CACHED_NC = build_nc()
    in_maps = shard_inputs(**inputs)
    res = run_bass_kernel_spmd(_CACHED_NC, in_maps, list(range(NC)))
    # Each core's "out" is a sum-shard of the output, column-rotated by
    # -i*DS. Un-rotate and sum to unshard.
    total = np.zeros((B, D), np.float32)
    for i in range(NC):
        total += np.roll(res.results[i]["out"], i * DS, axis=1)
    return total


if __name__ == "__main__":
    rng = np.random.default_rng(0)
    ins = {
        "x": rng.standard_normal((B, D), np.float32),
        "ada_emb": rng.standard_normal((B, ADA), np.float32),
        "base": rng.standard_normal((D, D), np.float32),
        "w1": rng.standard_normal((ADA, INTER), np.float32) / np.sqrt(ADA),
        "b1": rng.standard_normal((INTER,), np.float32) / np.sqrt(ADA),
        "w2": rng.standard_normal((INTER, D * RANK * 2), np.float32) / np.sqrt(INTER),
        "b2": rng.standard_normal((D * RANK * 2,), np.float32) / np.sqrt(INTER),
    }
    out = kernel(**ins)
    print("out", out.shape, out.dtype, float(np.abs(out).mean()))


# revision 36
# speedup vs baseline: 1.1691x; 1.0235x over previous
"""AdaLoRAWithBase distributed Trainium2 kernel (8 NeuronCores).

Reference computation (B=16, D=2048, ADA=1024, INTER=1024, R=8):
    h   = gelu(ada_emb @ w1 + b1)                  [B, INTER]
    xw  = h @ w2 + b2                              [B, 2*D*R]
    x_a = xw[:, :D*R]  -> [B, D, R]
    x_b = xw[:, D*R:]  -> [B, D, R]
    layer = base + einsum('bdr,bkr->bdk', x_a, x_b)
    out = x + einsum('bc,bco->bo', x, layer)

Key algebra: the B x D x D layer never needs to be materialized:
    out = x + x @ base + sum_r t[:, r] * x_b[:, :, r]
    t[b,r] = sum_d x[b,d] * x_a[b,d,r]

Distribution: RANK == n_cores == 8, so shard by rank r -- core i takes
the x_a and x_b columns of w2 belonging to rank i (stride-8 column
slices, 4096 of the 32768 columns = 1/8 of w2's 128MB). Each core then
computes, fully locally, with NO collectives:
  - h = gelu(ada_emb @ w1 + b1), replicated (w1 is 2MB bf16; measured on
    this runtime any collective costs 50-80us of launch-skew + cc-boot
    barrier, far more than the redundant compute),
  - t_i = sum over ALL d of x[:,d] * x_a[:,d,i]      (its own rank),
  - delta_i = t_i * x_b[:, :, i]                     [B, D], all of D,
  - (x + x @ base)[:, i*256:(i+1)*256]               (its base slice).
Core i's output is delta_i plus its base+residual slice: the output is
SUM-sharded and the host unshards by summing the 8 partials. Since SPMD
cores all run the identical program, the b-half columns are host-rotated
by -i*256 so each core's base slice lands at columns [0,256); the host
un-rotates with np.roll before summing.

All matmul operands are bf16 (1 cycle/row on the PE vs 4 for fp32, and
half the DMA bytes); PSUM accumulation and the t/delta arithmetic stay
fp32. Weight DMAs are spread across the sync (HWDGE) and gpsimd
(SWDGE) queues; the scalar engine only runs activations/copies.
"""

import sys

import numpy as np

for _p in ("/opt/trn_rl_repo",):
    if _p not in sys.path:
        sys.path.insert(0, _p)

from concourse import bacc, bass, mybir, tile
from concourse.bass_utils import run_bass_kernel_spmd

B, D, ADA, INTER, RANK = 16, 2048, 1024, 1024, 8
NC = 8
DS = D // NC          # 256: per-core base/residual o-slice
KA = ADA // 128       # 8 k-tiles for the h matmul
KH = INTER // 128     # 8 k-tiles (and m-tiles) for h / xw
KX = D // 128         # 16 k-tiles for the base matmul
F32 = mybir.dt.float32
BF16 = mybir.dt.bfloat16

_CACHED_NC = None


def build_nc():
    nc = bacc.Bacc(
        "TRN2",
        target_bir_lowering=False,
        debug=False,
        enable_asserts=False,
        num_devices=NC,
    )

    ada_d = nc.declare_dram_parameter("ada", [128, KA * B], BF16, isOutput=False)
    w1_d = nc.declare_dram_parameter("w1f", [128, KA * INTER], BF16, isOutput=False)
    b1_d = nc.declare_dram_parameter("b1r", [1, INTER], BF16, isOutput=False)
    # per-core: rank-i columns of w2, a-half then b-half, k-tiled
    w2_d = nc.declare_dram_parameter("w2s", [128, KH * 2 * D], BF16, isOutput=False)
    b2_d = nc.declare_dram_parameter("b2s", [1, 2 * D], BF16, isOutput=False)
    xt_d = nc.declare_dram_parameter("xts", [128, KX * B], BF16, isOutput=False)
    ones_d = nc.declare_dram_parameter("ones", [1, B], BF16, isOutput=False)
    xs_d = nc.declare_dram_parameter("xs", [B, D], F32, isOutput=False)
    base_d = nc.declare_dram_parameter("bases", [128, KX * DS], BF16, isOutput=False)
    ident_d = nc.declare_dram_parameter("ident", [B, B], BF16, isOutput=False)
    xsb_d = nc.declare_dram_parameter("xsb", [B, DS], BF16, isOutput=False)
    identf_d = nc.declare_dram_parameter("identf", [B, B], F32, isOutput=False)
    out_d = nc.declare_dram_parameter("out", [B, D], F32, isOutput=True)

    with tile.TileContext(nc) as tc:
        with (
            tc.tile_pool(name="const", bufs=1) as cpool,
            tc.tile_pool(name="w2p", bufs=6) as w2pool,
            tc.tile_pool(name="ps", bufs=6, space="PSUM") as pp,
        ):
            # ---- small input loads (sync queue) ------------------------
            ada_sb = cpool.tile([128, KA * B], BF16)
            nc.sync.dma_start(ada_sb[:], ada_d[:])
            w1_sb = cpool.tile([128, KA * INTER], BF16)
            half = KA * INTER // 2
            nc.sync.dma_start(w1_sb[:, :half], w1_d[:, :half])
            nc.gpsimd.dma_start(w1_sb[:, half:], w1_d[:, half:])
            b1_sb = cpool.tile([1, INTER], BF16)
            nc.sync.dma_start(b1_sb[:], b1_d[:])
            xs_sb = cpool.tile([B, D], F32)
            nc.sync.dma_start(xs_sb[:], xs_d[:])
            b2_sb = cpool.tile([1, 2 * D], BF16)
            nc.sync.dma_start(b2_sb[:], b2_d[:])
            ones_sb = cpool.tile([1, B], BF16)
            nc.sync.dma_start(ones_sb[:], ones_d[:])
            xt_sb = cpool.tile([128, KX * B], BF16)
            nc.sync.dma_start(xt_sb[:], xt_d[:])
            ident_sb = cpool.tile([B, B], BF16)
            nc.sync.dma_start(ident_sb[:], ident_d[:])
            xsb_sb = cpool.tile([B, DS], BF16)
            nc.sync.dma_start(xsb_sb[:], xsb_d[:])
            identf_sb = cpool.tile([B, B], F32)
            nc.sync.dma_start(identf_sb[:], identf_d[:])

            # ---- h = gelu(ada @ w1 + b1), replicated -------------------
            # [16,1024] orientation: tiny [128,16] weight loads, N=512
            # streams; then PE-transpose into hT k-tiles for the xw stage.
            ph = pp.tile([B, 1024], F32, tag="ps2", name="hpre", bufs=1)
            for nn in range(2):
                for k in range(KA):
                    nc.tensor.matmul(
                        ph[:, nn * 512 : (nn + 1) * 512],
                        ada_sb[:, k * B : (k + 1) * B],
                        w1_sb[:, k * INTER + nn * 512 : k * INTER + (nn + 1) * 512],
                        start=(k == 0),
                        stop=False,
                    )
                nc.tensor.matmul(  # b1 bias via K=1 ones matmul
                    ph[:, nn * 512 : (nn + 1) * 512],
                    ones_sb[:],
                    b1_sb[0:1, nn * 512 : (nn + 1) * 512],
                    start=False,
                    stop=True,
                )
            h_sb = cpool.tile([B, INTER], F32)
            ht_sb = cpool.tile([128, KH * B], BF16)
            for nn in range(2):  # per-half gelu+transpose: k<4 tiles ready early
                nc.scalar.activation(
                    h_sb[:, nn * 512 : (nn + 1) * 512],
                    ph[:, nn * 512 : (nn + 1) * 512],
                    mybir.ActivationFunctionType.Gelu,
                )
                for km in range(nn * 4, nn * 4 + 4):
                    pt = pp.tile([128, B], F32, tag="ps", name=f"pt{km}")
                    nc.tensor.transpose(
                        pt[:], h_sb[:, km * 128 : (km + 1) * 128], identf_sb[:]
                    )
                    nc.scalar.activation(
                        ht_sb[:, km * B : (km + 1) * B], pt[:],
                        mybir.ActivationFunctionType.Copy,
                    )

            # ---- base term + residual: x @ base[:, slice] + xs ---------
            base_ps = pp.tile([B, DS], F32, tag="ps", name="base_ps")
            bt = w2pool.tile([128, KX * DS], BF16, tag="w2", name="baset")
            nc.sync.dma_start(bt[:], base_d[:])
            for k in range(KX):
                nc.tensor.matmul(
                    base_ps[:],
                    xt_sb[:, k * B : (k + 1) * B],
                    bt[:, k * DS : (k + 1) * DS],
                    start=(k == 0),
                    stop=False,
                )
            nc.tensor.matmul(  # + residual: I^T @ xs == xs
                base_ps[:], ident_sb[:], xsb_sb[:], start=False, stop=True
            )

            # ---- xw a-half: n-major 1MB chunks, one PSUM bank each -----
            # chunk nb holds all 8 k-tiles for d-columns [nb*512,(nb+1)*512)
            psum_a = [pp.tile([B, 512], F32, tag="ps", name=f"psa{j}") for j in range(4)]
            psum_b = [pp.tile([B, 512], F32, tag="ps", name=f"psb{j}") for j in range(4)]
            tmp_t = cpool.tile([B, D], F32)
            t4 = cpool.tile([B, 4], F32)
            for nb in range(4):
                w2t = w2pool.tile([128, 2 * D], BF16, tag="w2", name=f"w2a{nb}")
                eng = nc.sync if nb % 2 == 0 else nc.gpsimd
                eng.dma_start(w2t[:], w2_d[:, nb * 4096 : (nb + 1) * 4096])
                for k in range(KH):
                    nc.tensor.matmul(
                        psum_a[nb][:],
                        ht_sb[:, k * B : (k + 1) * B],
                        w2t[:, k * 512 : (k + 1) * 512],
                        start=(k == 0),
                        stop=False,
                    )
                nc.tensor.matmul(  # b2 bias via K=1 ones matmul
                    psum_a[nb][:],
                    ones_sb[:],
                    b2_sb[0:1, nb * 512 : (nb + 1) * 512],
                    start=False,
                    stop=True,
                )
                # partial t for this bank, pipelined with the stream
                nc.vector.tensor_tensor(
                    tmp_t[:, nb * 512 : (nb + 1) * 512],
                    psum_a[nb][:],
                    xs_sb[:, nb * 512 : (nb + 1) * 512],
                    mybir.AluOpType.mult,
                )
                nc.vector.tensor_reduce(
                    t4[:, nb : nb + 1],
                    tmp_t[:, nb * 512 : (nb + 1) * 512],
                    axis=mybir.AxisListType.X, op=mybir.AluOpType.add,
                )
            t_sc = cpool.tile([B, 1], F32)
            nc.vector.tensor_reduce(
                t_sc[:], t4[:],
                axis=mybir.AxisListType.X, op=mybir.AluOpType.add,
            )

            # ---- xw b-half: n-major chunks, out muls pipelined ---------
            # last chunk split into two tiles so its matmuls overlap the
            # final 256KB of the stream (Tile deps are per-tile)
            out_sb = cpool.tile([B, D], F32)
            for nb in range(4):
                eng = nc.sync if nb % 2 == 0 else nc.gpsimd
                o = KH * D + nb * 4096
                if nb < 3:
                    w2t = w2pool.tile([128, 2 * D], BF16, tag="w2", name=f"w2b{nb}")
                    eng.dma_start(w2t[:], w2_d[:, o : o + 4096])
                    parts = [(w2t, 0, KH)]
                else:
                    w2t1 = w2pool.tile([128, 3072], BF16, tag="w2", name="w2b3a")
                    w2t2 = w2pool.tile([128, 1024], BF16, tag="w2tail", name="w2b3b")
                    eng.dma_start(w2t1[:], w2_d[:, o : o + 3072])
                    eng.dma_start(w2t2[:], w2_d[:, o + 3072 : o + 4096])
                    parts = [(w2t1, 0, 6), (w2t2, 6, KH)]
                for tile_, k0, k1 in parts:
                    for k in range(k0, k1):
                        nc.tensor.matmul(
                            psum_b[nb][:],
                            ht_sb[:, k * B : (k + 1) * B],
                            tile_[:, (k - k0) * 512 : (k - k0 + 1) * 512],
                            start=(k == 0),
                            stop=False,
                        )
                nc.tensor.matmul(
                    psum_b[nb][:],
                    ones_sb[:],
                    b2_sb[0:1, D + nb * 512 : D + (nb + 1) * 512],
                    start=False,
                    stop=True,
                )
                nc.vector.tensor_scalar_mul(
                    out_sb[:, nb * 512 : (nb + 1) * 512],
                    psum_b[nb][:],
                    t_sc[:, 0:1],
                )
                if nb == 0:  # base+residual lands in columns [0, DS)
                    nc.vector.tensor_tensor(
                        out_sb[:, 0:DS], out_sb[:, 0:DS], base_ps[:],
                        mybir.AluOpType.add,
                    )
                if nb == 2:  # stream out the finished 3/4 early
                    nc.sync.dma_start(out_d[:, 0:1536], out_sb[:, 0:1536])
            nc.sync.dma_start(out_d[:, 1536:], out_sb[:, 1536:])

    nc.compile()
    return nc


def _ktile(a: np.ndarray, p: int = 128) -> np.ndarray:
    """[K*p, m] -> [p, K*m] with free index = k*m + j (k-tile major)."""
    kp, m = a.shape
    k = kp // p
    return np.ascontiguousarray(
        a.reshape(k, p, m).transpose(1, 0, 2).reshape(p, k * m)
    )


def shard_inputs(x, ada_emb, base, w1, b1, w2, b2):
    import ml_dtypes

    bf16 = ml_dtypes.bfloat16
    x = np.ascontiguousarray(np.asarray(x, np.float32))
    ada_emb = np.asarray(ada_emb, np.float32)
    base = np.asarray(base, np.float32)
    w1 = np.asarray(w1, np.float32)
    b1 = np.asarray(b1, np.float32)
    w2 = np.asarray(w2, bf16)
    b2 = np.asarray(b2, np.float32)

    ada_pre = _ktile(np.ascontiguousarray(ada_emb.T)).astype(bf16)  # [128, 8*16]
    xt_pre = _ktile(np.ascontiguousarray(x.T)).astype(bf16)         # [128, 16*16]
    w1f = _ktile(w1).astype(bf16)                                  # [128, 8*1024]
    b1r = b1.reshape(1, INTER).astype(bf16)

    w2a, w2b = w2[:, : D * RANK], w2[:, D * RANK :]
    b2a, b2b = b2[: D * RANK], b2[D * RANK :]
    in_maps = []
    for i in range(NC):
        # rank-i columns: stride-RANK slices; b-half rotated by -i*DS so
        # the base/residual slice lands at output columns [0, DS)
        # n-major blocks: free idx = nb*(8*512) + k*512 + c
        def _nmajor(a):
            return np.ascontiguousarray(
                a.reshape(KH, 128, 4, 512).transpose(1, 2, 0, 3).reshape(128, KH * 2048)
            )
        w2ai = _nmajor(np.ascontiguousarray(w2a[:, i::RANK]))
        w2bi = _nmajor(np.ascontiguousarray(np.roll(w2b[:, i::RANK], -i * DS, axis=1)))
        in_maps.append({
            "ada": ada_pre,
            "w1f": w1f,
            "b1r": b1r,
            "w2s": np.ascontiguousarray(np.concatenate([w2ai, w2bi], axis=1)),
            "b2s": np.concatenate(
                [b2a[i::RANK], np.roll(b2b[i::RANK], -i * DS)]
            ).reshape(1, -1).astype(bf16),
            "ones": np.ones((1, B), bf16),
            "xts": xt_pre,
            "xs": x,
            "ident": np.eye(B, dtype=bf16),
            "identf": np.eye(B, dtype=np.float32),
            "xsb": np.ascontiguousarray(x[:, i * DS : (i + 1) * DS]).astype(bf16),
            "bases": _ktile(base[:, i * DS : (i + 1) * DS]).astype(bf16),
        })
    return in_maps


def kernel(**inputs) -> np.ndarray:
    global _CACHED_NC
    if _CACHED_NC is None:
        _CACHED_NC = build_nc()
    in_maps = shard_inputs(**inputs)
    res = run_bass_kernel_spmd(_CACHED_NC, in_maps, list(range(NC)))
    # Each core's "out" is a sum-shard of the output, column-rotated by
    # -i*DS. Un-rotate and sum to unshard.
    total = np.zeros((B, D), np.float32)
    for i in range(NC):
        total += np.roll(res.results[i]["out"], i * DS, axis=1)
    return total


if __name__ == "__main__":
    rng = np.random.default_rng(0)
    ins = {
        "x": rng.standard_normal((B, D), np.float32),
        "ada_emb": rng.standard_normal((B, ADA), np.float32),
        "base": rng.standard_normal((D, D), np.float32),
        "w1": rng.standard_normal((ADA, INTER), np.float32) / np.sqrt(ADA),
        "b1": rng.standard_normal((INTER,), np.float32) / np.sqrt(ADA),
        "w2": rng.standard_normal((INTER, D * RANK * 2), np.float32) / np.sqrt(INTER),
        "b2": rng.standard_normal((D * RANK * 2,), np.float32) / np.sqrt(INTER),
    }
    out = kernel(**ins)
    print("out", out.shape, out.dtype, float(np.abs(out).mean()))
